# revision 61
# baseline (speedup 1.0000x reference)
"""Block-sparse attention Trainium2 kernel (8 NeuronCores, SPMD).

Sharding: data-parallel over (batch, head-group): core c handles batch b=c//4
and heads [4*(c%4) .. 4*(c%4)+4). Block index lists are replicated (used
host-side to build the static program). Each core returns a partial
[S, E] output (its heads' contribution through Wo); the host sums the 4
partials per batch and adds bo once.

Design (transposed-scores dataflow):
  Host feeds x^T and W_qkv in bf16. QKV projection runs weights-stationary
  (bf16, 1 cyc/row at 512-wide moving) producing q^T,k^T (bf16, q
  pre-scaled by 1/sqrt(D)) and v^T (bf16); V goes to [keys, d] layout via
  PE transposes. Per head pair, block-diagonal stationaries are built by
  SBUF->SBUF DMA:
    kdiag_j = [[kT_A(j), 0], [0, kT_B(j)]]   (dA|dB x keysA|keysB)
    Vdiag_j = [[V_A(j), 0], [0, V_B(j)]]     (keysA|keysB x dA|dB)
  Active (i,j) blocks are bin-packed into chunks of <=8 row-block pieces
  (<=512 cols, one PSUM bank), separately per i-QUARTER so each chunk's
  phase-B accumulates into a single-bank out^T tile (8 segments total:
  4 quarters x 2 head pairs, otp ring of 2 so the flush copy overlaps the
  next segment).  Per chunk, three stages:
    A: scoresT = kdiag_j^T @ qT[:, i-cols] -> PSUM; exp -> SBUF bf16 (ACT)
    B: den = onesdiag^T @ expT -> PSUM; rec = recip(den) (DVE);
       at2 = ex*rec (mult split DVE:GPSIMD 5:11)
    C: otp += Vdiag_j^T @ at2
  Phase 2 software-pipelines A/B/C in the PE stream (A(p+5)/B(p+3)/C(p)
  minimum skew, plus opportunistic deep lookahead bounded by the ex ring
  and at pool, with B kept >=2 behind A) so the
  tensor engine rarely waits on the ACT/DVE/GPSIMD front-end chain; the
  QKV phase interleaves A/B of early chunks between projection blocks
  (hybrid order: k/q/vA per arriving x chunk, then pair-1 projections).
  The Wo projection (float32r, PSUM ring shared with scores) is
  interleaved into phase 2: quarter q's four Wo column blocks unlock at
  the segment-(4+q) flush and are spread one per three C-steps, filling
  PE while the saturated DVE/GPSIMD front-end drains; y is DMA'd per
  128-row tile on the sync queue.  Startup: x/w DMAs are split per
  k-tile on the two hardware-DGE queues (sync/ACT) in consumption order
  so the first projection matmul starts as soon as the first slices
  land; the big diag memsets are emitted after the triggers.  rec is
  written bf16 (custom-DVE recip with bf16 output) to halve the mult's
  SBUF read traffic.  Bias is added on the host.
"""
import numpy as np

B, S, E, H, D, BS = 2, 2048, 1024, 16, 64, 64
NB = S // BS          # 32
NCORES = 8
HPC = 4               # heads per core

LAST_RESULTS = None   # BassKernelResults of the most recent run (for test.py)


# ---------------------------------------------------------------- host planning

def _plan(block_rows, block_cols):
    """j-major static schedule shared by every head-pair/core.

    For each col-block j: active row-blocks i, split into maximal
    consecutive runs that do not cross multiples of 8 (PSUM bank alignment
    for the out^T accumulator), grouped into chunks of <=8 blocks
    (<=512 cols, one PSUM bank per scores/den tile), packed per i-quarter.

    Returns:
      chunks: list of (j, [(pos, i0, n), ...]) in emission order; pos is the
              64-col block offset inside the chunk tile.
      flags:  dict (j, i0) -> [start, stop] for the otp accumulation.
      qbound: chunk-index boundaries of the 4 i-quarters (len 5).
    """
    mask = np.zeros((NB, NB), dtype=bool)
    for r, c in zip(np.asarray(block_rows).tolist(), np.asarray(block_cols).tolist()):
        mask[int(r), int(c)] = True

    pieces = []          # (j, i0, n) in j-major order
    for j in range(NB):
        ilist = np.nonzero(mask[:, j])[0].tolist()
        cur = None
        for i in ilist:
            if cur is not None and i == cur[1] + cur[2] and (i % 8 != 0):
                cur[2] += 1
            else:
                cur = [j, i, 1]
                pieces.append(cur)
    # bin-pack pieces into chunks of <=8 blocks (first-fit decreasing)
    def pack(plist):
        bins = []
        for j, i0, n in sorted(plist, key=lambda p: -p[2]):
            for b in bins:
                if b[0] + n <= 8:
                    b[0] += n
                    b[1].append((j, i0, n))
                    break
            else:
                bins.append([n, [(j, i0, n)]])
        for b in bins:
            b[1].sort()
        bins.sort(key=lambda b: b[1][0])
        out = []
        for _, pl in bins:
            group, nb_ = [], 0
            for j, i0, n in pl:
                group.append((nb_, j, i0, n))
                nb_ += n
            out.append(group)
        return out
    chunks = []
    qbound = [0]
    for q in range(4):
        chunks.extend(pack([p for p in pieces if p[1] // 8 == q]))
        qbound.append(len(chunks))
    # otp start/stop: first/last piece per 8-i bank in emission order
    flags = {}
    first_seen, last_seen = {}, {}
    for group in chunks:
        for pos, j, i0, n in group:
            bk = i0 // 8
            assert (i0 + n - 1) // 8 == bk
            if bk not in first_seen:
                first_seen[bk] = (j, i0)
            last_seen[bk] = (j, i0)
            flags[(j, i0)] = [False, False]
    for bk, key in first_seen.items():
        flags[key][0] = True
    for bk, key in last_seen.items():
        flags[key][1] = True
    return dict(chunks=chunks, flags=flags, qbound=qbound)


# ---------------------------------------------------------------- bass program

def _build_program(plan, nsplit=44, dve_mult=(0, 3, 6, 9, 12), la=5, lb=3):
    import concourse.bacc as bacc
    import concourse.mybir as mybir
    from concourse.tile import TileContext
    from concourse import masks

    from concourse.dve_ops import (
        RECIP_APPROX_FAST_CONSTS as _RC,
        RECIPROCAL_APPROX_FAST as _RF,
    )

    F32 = mybir.dt.float32
    F32R = mybir.dt.float32r
    BF16 = mybir.dt.bfloat16
    AF = mybir.ActivationFunctionType
    ALU = mybir.AluOpType

    nc = bacc.Bacc("TRN2", target_bir_lowering=False, debug=False)

    xT_in = nc.dram_tensor("xT_local", [E, S], BF16, kind="ExternalInput")
    wqkv_in = nc.dram_tensor("w_qkv", [E, 3 * HPC * D], BF16, kind="ExternalInput")
    bqkv_in = nc.dram_tensor("b_qkv", [3 * HPC * D], F32, kind="ExternalInput")
    wo_in = nc.dram_tensor("w_o", [HPC * D, E], F32R, kind="ExternalInput")
    y_out = nc.dram_tensor("y_partial", [S, E], BF16, kind="ExternalOutput")

    NT = 3 * HPC * D // 128      # 6 qkv n-tiles
    KT = E // 128                # 8 contraction tiles
    ST = S // 128                # 16 s tiles
    SC = S // 512                # 4 s-chunks

    chunks, flags = plan['chunks'], plan['flags']
    qbound = plan['qbound']
    nch = len(chunks)

    with TileContext(nc) as tc:
        with tc.tile_pool(name="const", bufs=1) as cpool, \
             tc.tile_pool(name="qk", bufs=1) as qkpool, \
             tc.tile_pool(name="vt", bufs=1) as vtpool, \
             tc.tile_pool(name="diag", bufs=1) as dgpool, \
             tc.tile_pool(name="outsb", bufs=1) as opool, \
             tc.tile_pool(name="wo", bufs=1) as wop:

            idb = cpool.tile([128, 128], BF16)
            masks.make_identity(nc, idb[:])
            bqkv_sb = cpool.tile([128, NT], F32)
            nc.scalar.dma_start(bqkv_sb[:],
                                bqkv_in.ap().rearrange("(t p) -> p t", p=128))
            bsc = cpool.tile([128, NT], F32)
            nc.scalar.mul(bsc[:, 0:2], bqkv_sb[:, 0:2], 0.125)
            nc.scalar.copy(bsc[:, 2:NT], bqkv_sb[:, 2:NT])
            onesdiag = cpool.tile([128, 128], BF16)
            nc.gpsimd.memset(onesdiag[:], 0.0)
            nc.gpsimd.memset(onesdiag[0:64, 0:64], 1.0)
            nc.gpsimd.memset(onesdiag[64:128, 64:128], 1.0)

            wo_sb = [wop.tile([128, E], F32R, name=f"wo{hp}") for hp in range(2)]
            qT = [qkpool.tile([128, S], BF16, name=f"qT{hp}") for hp in range(2)]
            kT = [qkpool.tile([128, S], BF16, name=f"kT{hp}") for hp in range(2)]
            V = [vtpool.tile([128, (NB // 2) * D], BF16, name=f"V{h}")
                 for h in range(HPC)]
            kdiag = [dgpool.tile([128, NB * 128], BF16, name=f"kdiag{hp}")
                     for hp in range(2)]
            Vdiag = [dgpool.tile([128, NB * 128], BF16, name=f"Vdiag{hp}")
                     for hp in range(2)]
            outSB = [opool.tile([128, S], F32R, name=f"outSB{hp}") for hp in range(2)]

            with tc.tile_pool(name="sc_ps", bufs=3, space="PSUM") as sc_ps, \
                 tc.tile_pool(name="dn_ps", bufs=2, space="PSUM") as dn_ps, \
                 tc.tile_pool(name="ex", bufs=6) as expool, \
                 tc.tile_pool(name="rc", bufs=6) as rcpool, \
                 tc.tile_pool(name="at", bufs=nsplit + 14) as atpool:

                ex_saved = {}    # (hp, ci) -> (ex tile, ncols)
                at2_saved = {}   # (hp, ci) -> at2 tile
                mult_ctr = [0]
                dn_pools = [dn_ps]   # phase 2 appends a 3rd ring slot

                def emit_A(hp, ci):
                    """scores -> exp for one chunk."""
                    group = chunks[ci]
                    ncols = sum(n for _, _, _, n in group) * 64
                    spt = sc_ps.tile([128, 512], F32, tag="spt")
                    for gi, (pos, j, i0, n) in enumerate(group):
                        nc.tensor.matmul(
                            spt[:, pos * 64:(pos + n) * 64],
                            kdiag[hp][:, j * 128:(j + 1) * 128],
                            qT[hp][:, i0 * 64:(i0 + n) * 64],
                            start=(gi == 0), stop=(gi == len(group) - 1))
                    ex = expool.tile([128, 512], BF16, tag="ex")
                    nc.scalar.activation(ex[:, 0:ncols], spt[:, 0:ncols], AF.Exp)
                    ex_saved[(hp, ci)] = (ex, ncols)

                def emit_B(hp, ci):
                    """den -> recip -> mult for one chunk (needs A done)."""
                    ex, ncols = ex_saved.pop((hp, ci))
                    pool = (dn_pools[1] if len(dn_pools) > 1
                            and mult_ctr[0] % 3 == 2 else dn_pools[0])
                    dnb = pool.tile([128, 512], F32, tag="dnb")
                    nc.tensor.matmul(dnb[:, 0:ncols], onesdiag[:],
                                     ex[:, 0:ncols], start=True, stop=True)
                    # bf16 rec halves SBUF read traffic of the mult; the
                    # custom op computes in fp32 and casts on write (the
                    # fp32-only wrapper assert is about the INPUT bit trick)
                    rec = rcpool.tile([128, 512], BF16, tag="rec")
                    nc.vector._custom_dve(_RF, out=rec[:, 0:ncols],
                                          in0=dnb[:, 0:ncols], s0=_RC["s0"],
                                          s1=_RC["s1"], imm2=_RC["imm2"])
                    at2 = atpool.tile([128, 512], BF16, tag="at2")
                    k = mult_ctr[0] % 16
                    mult_ctr[0] += 1
                    if k in dve_mult:
                        nc.vector.tensor_tensor(at2[:, 0:ncols], ex[:, 0:ncols],
                                                rec[:, 0:ncols], ALU.mult)
                    else:
                        nc.gpsimd.tensor_tensor(at2[:, 0:ncols], ex[:, 0:ncols],
                                                rec[:, 0:ncols], ALU.mult)
                    at2_saved[(hp, ci)] = at2

                def emit_C(hp, ci, otp, ibase):
                    at2 = at2_saved.pop((hp, ci))
                    for pos, j, i0, n in chunks[ci]:
                        st, sp = flags[(j, i0)]
                        o0 = (i0 - ibase) * 64
                        nc.tensor.matmul(
                            otp[:, o0:o0 + n * 64],
                            Vdiag[hp][:, j * 128:(j + 1) * 128],
                            at2[:, pos * 64:(pos + n) * 64],
                            start=st, stop=sp)

                # ---- QKV (f32-free: bf16 weights-stationary) ----------------
                qkv_scale = [0.125, 0.125, 1.0, 1.0, 1.0, 1.0]
                with tc.tile_pool(name="xin", bufs=1) as xpool, \
                     tc.tile_pool(name="wq", bufs=1) as wpool, \
                     tc.tile_pool(name="qkv_ps", bufs=2, space="PSUM") as qkv_ps, \
                     tc.tile_pool(name="tr_ps", bufs=1, space="PSUM") as tr_ps:
                    vT = [xpool.tile([128, S], BF16, name=f"vT{hp}")
                          for hp in range(2)]
                    qkv_dst = [qT[0], qT[1], kT[0], kT[1], vT[0], vT[1]]
                    wsb = [wpool.tile([128, 3 * HPC * D], BF16, name=f"w{k}")
                           for k in range(KT)]
                    xsc = [xpool.tile([128, KT, 512], BF16, name=f"xsc{sc}")
                           for sc in range(SC)]
                    dma_engs = (nc.sync, nc.scalar)   # hw-DGE queues only
                    # weights first (consumed by every block), then x slices
                    # per k-tile in consumption order.
                    for k in range(KT):
                        dma_engs[k % 2].dma_start(
                            wsb[k][:], wqkv_in.ap()[k * 128:(k + 1) * 128, :])
                    xT_v = xT_in.ap().rearrange("(k p) s -> p k s", p=128)
                    for sc in range(SC):
                        for k in range(KT):
                            dma_engs[(sc * KT + k) % 2].dma_start(
                                xsc[sc][:, k, :],
                                xT_v[:, k, sc * 512:(sc + 1) * 512])
                    # big diag zero-fills: after the DMA triggers so they
                    # don't delay the x/w streams
                    nc.vector.memset(kdiag[0][:], 0.0)
                    nc.gpsimd.memset(kdiag[1][:], 0.0)
                    nc.vector.memset(Vdiag[0][:], 0.0)
                    nc.gpsimd.memset(Vdiag[1][:], 0.0)

                    def emit_qkv_block(t, sc):
                        pt = qkv_ps.tile([128, 512], F32, tag="qkvmm")
                        for k in range(KT):
                            nc.tensor.matmul(
                                pt[:],
                                wsb[k][:, t * 128:(t + 1) * 128],
                                xsc[sc][:, k, :],
                                start=(k == 0), stop=(k == KT - 1))
                        nc.scalar.activation(
                            qkv_dst[t][:, sc * 512:(sc + 1) * 512], pt[:],
                            AF.Identity, bias=bsc[:, t:t + 1],
                            scale=qkv_scale[t])

                    def emit_kdiag(hp, quarter=None):
                        kd = kdiag[hp][:, :]
                        for (p0, c0) in ((0, 0), (64, 64)):
                            dst = kd[p0:p0 + 64, :].rearrange(
                                "p (j c) -> p j c", c=128)[:, :, c0:c0 + 64]
                            src = kT[hp][p0:p0 + 64, :].rearrange(
                                "p (j c) -> p j c", c=64)
                            if quarter is None:
                                nc.sync.dma_start(dst, src)
                            else:
                                q8 = quarter * 8
                                nc.sync.dma_start(dst[:, q8:q8 + 8, :],
                                                  src[:, q8:q8 + 8, :])

                    def emit_vprep(vp):
                        for c4 in range(0, NB // 2, 4):
                            tp = tr_ps.tile([128, 512], BF16, tag="vtr")
                            for u in range(4):
                                c = c4 + u
                                nc.tensor.transpose(
                                    tp[:, u * 128:(u + 1) * 128],
                                    vT[vp][:, c * 128:(c + 1) * 128], idb[:])
                            for lh in range(2):
                                src = tp[:, 0:512].rearrange(
                                    "p (u x) -> p u x", x=128)[
                                    :, :, lh * 64:(lh + 1) * 64]
                                dst = V[2 * vp + lh][
                                    :, c4 * 64:(c4 + 4) * 64].rearrange(
                                    "p (u d) -> p u d", d=64)
                                if lh == 0:
                                    nc.scalar.copy(dst, src)
                                else:
                                    nc.vector.tensor_copy(dst, src)
                        vd = Vdiag[vp][:, :]
                        for lh in range(2):
                            h = 2 * vp + lh
                            pd, cd = (0, 0) if lh == 0 else (64, 64)
                            for par in range(2):
                                dst = vd[pd:pd + 64, :].rearrange(
                                    "p (c x) -> p c x", x=256)[
                                    :, :,
                                    par * 128 + cd:par * 128 + cd + 64]
                                src = V[h][par * 64:(par + 1) * 64, :].rearrange(
                                    "p (c d) -> p c d", d=64)
                                nc.sync.dma_start(dst, src)

                    # chunk ci eligible once its qT s-chunk (== its i-quarter)
                    # and kdiag quarters are written (subtile deps enforce
                    # correctness; this ordering only aids overlap)
                    nfront = min(nsplit, nch)
                    need_q = [max((i0 + n - 1) // 8 for _, _, i0, n in
                                  chunks[ci]) for ci in range(nfront)]
                    need_k = [max(j // 8 for _, j, _, _ in chunks[ci])
                              for ci in range(nfront)]
                    next_A = [0]
                    pend_B = []

                    def emit_eligible(sdone, kdone, cap):
                        # drain pending B's (their exp had >=1 block of PE
                        # time to complete), keeping one in flight, then
                        # emit new A's
                        done = 0
                        while len(pend_B) > 1 and done < cap:
                            emit_B(0, pend_B.pop(0))
                            done += 1
                        while (next_A[0] < nfront and done < cap
                               and need_q[next_A[0]] < sdone
                               and need_k[next_A[0]] < kdone):
                            emit_A(0, next_A[0])
                            pend_B.append(next_A[0])
                            next_A[0] += 1
                            done += 1

                    # sc-major order: all 6 projections of an x chunk run
                    # before the next chunk is needed, so PE never waits on
                    # a later x DMA while compute on arrived data remains
                    # hybrid order: q/k/vA per x chunk as it arrives (keeps
                    # PE fed during the DMA stream and unlocks fronts per
                    # quarter), then the pair-1 projections with fronts
                    # interleaved
                    for sc in range(SC):
                        emit_qkv_block(2, sc)
                        emit_kdiag(0, quarter=sc)
                        emit_eligible(sc, sc + 1, 4)
                        emit_qkv_block(0, sc)
                        emit_eligible(sc + 1, sc + 1, 8)
                        emit_qkv_block(4, sc)
                        emit_eligible(sc + 1, sc + 1, 4)
                    emit_vprep(0)
                    blocks_rest = [(t, sc) for t in (1, 3, 5)
                                   for sc in range(SC)]
                    bi_ = 0

                    def drain_block():
                        nonlocal bi_
                        t, sc = blocks_rest[bi_]
                        emit_qkv_block(t, sc)
                        bi_ += 1
                        if (t, sc) == (3, SC - 1):
                            emit_kdiag(1)
                        elif (t, sc) == (5, SC - 1):
                            emit_vprep(1)

                    while next_A[0] < nfront or pend_B or bi_ < len(blocks_rest):
                        emit_eligible(SC, SC, 3)
                        if bi_ < len(blocks_rest):
                            drain_block()
                        elif next_A[0] < nfront:
                            emit_A(0, next_A[0])
                            pend_B.append(next_A[0])
                            next_A[0] += 1
                        elif pend_B:
                            emit_B(0, pend_B.pop(0))

                for hp in range(2):
                    eng = nc.scalar if hp else nc.sync
                    eng.dma_start(wo_sb[hp][:],
                                  wo_in.ap()[hp * 128:(hp + 1) * 128, :])

                # ---- attention phase B: software-pipelined A/B/C ------------
                # flat schedule over (hp, ci); segment = one 1-bank otp pass
                sched = []
                seg_of = []
                seg_ibase = []
                seg_wo_q = []   # wo quarter unlocked when this seg flushes
                for hp in range(2):
                    for q in range(4):
                        sid = len(seg_ibase)
                        seg_ibase.append(q * 8)
                        seg_wo_q.append(q if hp == 1 else None)
                        for ci in range(qbound[q], qbound[q + 1]):
                            sched.append((hp, ci))
                            seg_of.append(sid)
                npos = len(sched)
                a_cur = [0]
                b_cur = [0]

                atcap = nsplit + 8   # at-pool bufs minus in-flight margin

                def ensure_A(upto):
                    # advance to `upto` unconditionally, then keep going
                    # while the ex ring and at pool have room -- keeps the
                    # front-end fed while pre-saved at2 chunks drain
                    while a_cur[0] < npos:
                        hp, ci = sched[a_cur[0]]
                        done = (hp, ci) in ex_saved or (hp, ci) in at2_saved
                        if not done:
                            if a_cur[0] > upto and (
                                    len(ex_saved) >= 5
                                    or len(at2_saved) >= atcap):
                                break
                            emit_A(hp, ci)
                        a_cur[0] += 1

                def ensure_B(upto):
                    # opportunistic B stays >=2 behind A so a fresh den
                    # never waits on an exp emitted in the same burst
                    lim = npos if a_cur[0] >= npos else a_cur[0] - 2
                    while b_cur[0] < min(lim, npos):
                        hp, ci = sched[b_cur[0]]
                        if (hp, ci) in ex_saved:
                            if b_cur[0] > upto and len(at2_saved) >= atcap:
                                break
                            emit_B(hp, ci)
                        b_cur[0] += 1

                # Wo blocks for i-quarter q need outSB[0] and outSB[1] cols
                # q*512:(q+1)*512, i.e. the flushes of segments q and 4+q --
                # emit them right after segment 4+q flushes so the Wo matmuls
                # fill PE while the saturated DVE/GPSIMD front-end drains.
                # The Wo PSUM tile shares the sc_ps ring (same tag/shape).
                with tc.tile_pool(name="yt", bufs=4) as ypool, \
                     tc.tile_pool(name="ot_ps", bufs=2, space="PSUM") as ot_ps, \
                     tc.tile_pool(name="dn2_ps", bufs=1, space="PSUM") as dn2_ps:
                    dn_pools.append(dn2_ps)   # den ring 2 -> 3 for phase 2

                    wo_pend = []

                    def emit_wo_block(st_):
                        yt = ypool.tile([128, E], BF16, tag="yt", name="yt")
                        for nchk in range(2):
                            pt = sc_ps.tile([128, 512], F32, tag="spt")
                            for hp in range(2):
                                nc.tensor.matmul(
                                    pt[:],
                                    outSB[hp][:, st_ * 128:(st_ + 1) * 128],
                                    wo_sb[hp][:, nchk * 512:(nchk + 1) * 512],
                                    start=(hp == 0), stop=(hp == 1))
                            nc.scalar.copy(
                                yt[:, nchk * 512:(nchk + 1) * 512], pt[:])
                        nc.sync.dma_start(
                            y_out.ap()[st_ * 128:(st_ + 1) * 128, :], yt[:])

                    def seg_done(seg, hp_prev):
                        # hp1 segments flush on DVE: ACT carries exp + the
                        # yt copies of the interleaved Wo blocks
                        _flush_otp(nc, outSB, otp, seg_ibase[seg], hp_prev,
                                   on_act=(seg_wo_q[seg] is None))
                        if seg_wo_q[seg] is not None:
                            q = seg_wo_q[seg]
                            wo_pend.extend(range(4 * q, 4 * q + 4))

                    otp = None
                    cur_seg = -1
                    for p in range(npos):
                        ensure_A(p + la)
                        ensure_B(p + lb)
                        if wo_pend and p % 3 == 0:
                            emit_wo_block(wo_pend.pop(0))
                        if seg_of[p] != cur_seg:
                            if otp is not None:
                                seg_done(cur_seg, sched[p - 1][0])
                            cur_seg = seg_of[p]
                            otp = ot_ps.tile([128, 512], F32, tag="otp")
                        hp, ci = sched[p]
                        emit_C(hp, ci, otp, seg_ibase[cur_seg])
                    seg_done(cur_seg, sched[npos - 1][0])
                    for st_ in wo_pend:
                        emit_wo_block(st_)

    nc.compile()
    return nc


def _flush_otp(nc, outSB, otp, ibase, hp, on_act=True):
    dst = outSB[hp][:, ibase * 64:ibase * 64 + 512]
    if on_act:
        nc.scalar.copy(dst, otp[:, 0:512])
    else:
        nc.vector.tensor_copy(dst, otp[:, 0:512])


# ---------------------------------------------------------------- entry point

def kernel(x, Wq, bq, Wk, bk, Wv, bv, Wo, bo, block_rows, block_cols):
    global LAST_RESULTS
    from concourse.bass_utils import run_bass_kernel_spmd
    import os

    x = np.asarray(x, dtype=np.float32)
    Wq, Wk, Wv, Wo = (np.asarray(a, dtype=np.float32) for a in (Wq, Wk, Wv, Wo))
    bq, bk, bv, bo = (np.asarray(a, dtype=np.float32) for a in (bq, bk, bv, bo))

    plan = _plan(block_rows, block_cols)
    nc = _build_program(plan)

    import ml_dtypes
    bf16 = ml_dtypes.bfloat16
    xT = [np.ascontiguousarray(x[b].T).astype(bf16) for b in range(B)]
    in_maps = []
    for c in range(NCORES):
        b, g = c // 4, c % 4
        cs = slice(g * HPC * D, (g + 1) * HPC * D)
        w_qkv = np.ascontiguousarray(
            np.concatenate([Wq[:, cs], Wk[:, cs], Wv[:, cs]], axis=1)).astype(bf16)
        b_qkv = np.ascontiguousarray(
            np.concatenate([bq[cs], bk[cs], bv[cs]]))
        w_o = np.ascontiguousarray(Wo[cs, :])
        in_maps.append(dict(xT_local=xT[b], w_qkv=w_qkv, b_qkv=b_qkv, w_o=w_o))

    trace = bool(int(os.environ.get("KERNEL_TRACE", "0")))
    res = run_bass_kernel_spmd(nc, in_maps, core_ids=list(range(NCORES)),
                               trace=trace)
    LAST_RESULTS = res

    y = np.zeros((B, S, E), dtype=np.float32)
    for c in range(NCORES):
        y[c // 4] += np.asarray(res.results[c]["y_partial"], dtype=np.float32)
    y += bo
    return y


# revision 62
# speedup vs baseline: 1.0108x; 1.0108x over previous
"""Block-sparse attention Trainium2 kernel (8 NeuronCores, SPMD).

Sharding: data-parallel over (batch, head-group): core c handles batch b=c//4
and heads [4*(c%4) .. 4*(c%4)+4). Block index lists are replicated (used
host-side to build the static program). Each core returns a partial
[S, E] output (its heads' contribution through Wo); the host sums the 4
partials per batch and adds bo once.

Design (transposed-scores dataflow):
  Host feeds x^T and W_qkv in bf16. QKV projection runs weights-stationary
  (bf16, 1 cyc/row at 512-wide moving) producing q^T,k^T (bf16, q
  pre-scaled by 1/sqrt(D)) and v^T (bf16); V goes to [keys, d] layout via
  PE transposes. Per head pair, block-diagonal stationaries are built by
  SBUF->SBUF DMA:
    kdiag_j = [[kT_A(j), 0], [0, kT_B(j)]]   (dA|dB x keysA|keysB)
    Vdiag_j = [[V_A(j), 0], [0, V_B(j)]]     (keysA|keysB x dA|dB)
  Active (i,j) blocks are bin-packed into chunks of <=8 row-block pieces
  (<=512 cols, one PSUM bank), separately per i-QUARTER so each chunk's
  phase-B accumulates into a single-bank out^T tile (8 segments total:
  4 quarters x 2 head pairs, otp ring of 2 so the flush copy overlaps the
  next segment).  Per chunk, three stages:
    A: scoresT = kdiag_j^T @ qT[:, i-cols] -> PSUM; exp -> SBUF bf16 (ACT)
    B: den = onesdiag^T @ expT -> PSUM; rec = recip(den) (DVE);
       at2 = ex*rec (mult split DVE:GPSIMD 5:11)
    C: otp += Vdiag_j^T @ at2
  Phase 2 software-pipelines A/B/C in the PE stream (A(p+5)/B(p+3)/C(p)
  minimum skew, plus opportunistic deep lookahead bounded by the ex ring
  and at pool, with B kept >=2 behind A) so the
  tensor engine rarely waits on the ACT/DVE/GPSIMD front-end chain; the
  QKV phase interleaves A/B of early chunks between projection blocks
  (hybrid order: k/q/vA per arriving x chunk, then pair-1 projections).
  The Wo projection (float32r, PSUM ring shared with scores) is
  interleaved into phase 2: quarter q's four Wo column blocks unlock at
  the segment-(4+q) flush and are spread one per three C-steps, filling
  PE while the saturated DVE/GPSIMD front-end drains; y is DMA'd per
  128-row tile on the sync queue.  Startup: x/w DMAs are split per
  k-tile on the two hardware-DGE queues (sync/ACT) in consumption order
  so the first projection matmul starts as soon as the first slices
  land; the big diag memsets are emitted after the triggers.  rec is
  written bf16 (custom-DVE recip with bf16 output) to halve the mult's
  SBUF read traffic.  Bias is added on the host.
"""
import numpy as np

B, S, E, H, D, BS = 2, 2048, 1024, 16, 64, 64
NB = S // BS          # 32
NCORES = 8
HPC = 4               # heads per core

LAST_RESULTS = None   # BassKernelResults of the most recent run (for test.py)


# ---------------------------------------------------------------- host planning

def _plan(block_rows, block_cols):
    """j-major static schedule shared by every head-pair/core.

    For each col-block j: active row-blocks i, split into maximal
    consecutive runs that do not cross multiples of 8 (PSUM bank alignment
    for the out^T accumulator), grouped into chunks of <=8 blocks
    (<=512 cols, one PSUM bank per scores/den tile), packed per i-quarter.

    Returns:
      chunks: list of (j, [(pos, i0, n), ...]) in emission order; pos is the
              64-col block offset inside the chunk tile.
      flags:  dict (j, i0) -> [start, stop] for the otp accumulation.
      qbound: chunk-index boundaries of the 4 i-quarters (len 5).
    """
    mask = np.zeros((NB, NB), dtype=bool)
    for r, c in zip(np.asarray(block_rows).tolist(), np.asarray(block_cols).tolist()):
        mask[int(r), int(c)] = True

    pieces = []          # (j, i0, n) in j-major order
    for j in range(NB):
        ilist = np.nonzero(mask[:, j])[0].tolist()
        cur = None
        for i in ilist:
            if cur is not None and i == cur[1] + cur[2] and (i % 8 != 0):
                cur[2] += 1
            else:
                cur = [j, i, 1]
                pieces.append(cur)
    # bin-pack pieces into chunks of <=8 blocks (first-fit decreasing)
    def pack(plist):
        bins = []
        for j, i0, n in sorted(plist, key=lambda p: -p[2]):
            for b in bins:
                if b[0] + n <= 8:
                    b[0] += n
                    b[1].append((j, i0, n))
                    break
            else:
                bins.append([n, [(j, i0, n)]])
        for b in bins:
            b[1].sort()
        bins.sort(key=lambda b: b[1][0])
        out = []
        for _, pl in bins:
            group, nb_ = [], 0
            for j, i0, n in pl:
                group.append((nb_, j, i0, n))
                nb_ += n
            out.append(group)
        return out
    chunks = []
    qbound = [0]
    for q in range(4):
        chunks.extend(pack([p for p in pieces if p[1] // 8 == q]))
        qbound.append(len(chunks))
    # otp start/stop: first/last piece per 8-i bank in emission order
    flags = {}
    first_seen, last_seen = {}, {}
    for group in chunks:
        for pos, j, i0, n in group:
            bk = i0 // 8
            assert (i0 + n - 1) // 8 == bk
            if bk not in first_seen:
                first_seen[bk] = (j, i0)
            last_seen[bk] = (j, i0)
            flags[(j, i0)] = [False, False]
    for bk, key in first_seen.items():
        flags[key][0] = True
    for bk, key in last_seen.items():
        flags[key][1] = True
    return dict(chunks=chunks, flags=flags, qbound=qbound)


# ---------------------------------------------------------------- bass program

def _build_program(plan, nsplit=44, dve_mult=(0, 3, 6, 9, 12), la=5, lb=3):
    import concourse.bacc as bacc
    import concourse.mybir as mybir
    from concourse.tile import TileContext
    from concourse import masks

    from concourse.dve_ops import (
        RECIP_APPROX_FAST_CONSTS as _RC,
        RECIPROCAL_APPROX_FAST as _RF,
    )

    F32 = mybir.dt.float32
    F32R = mybir.dt.float32r
    BF16 = mybir.dt.bfloat16
    AF = mybir.ActivationFunctionType
    ALU = mybir.AluOpType

    nc = bacc.Bacc("TRN2", target_bir_lowering=False, debug=False)

    xT_in = nc.dram_tensor("xT_local", [E, S], BF16, kind="ExternalInput")
    wqkv_in = nc.dram_tensor("w_qkv", [E, 3 * HPC * D], BF16, kind="ExternalInput")
    bqkv_in = nc.dram_tensor("b_qkv", [3 * HPC * D], F32, kind="ExternalInput")
    wo_in = nc.dram_tensor("w_o", [HPC * D, E], F32R, kind="ExternalInput")
    y_out = nc.dram_tensor("y_partial", [S, E], BF16, kind="ExternalOutput")

    NT = 3 * HPC * D // 128      # 6 qkv n-tiles
    KT = E // 128                # 8 contraction tiles
    ST = S // 128                # 16 s tiles
    SC = S // 512                # 4 s-chunks

    chunks, flags = plan['chunks'], plan['flags']
    qbound = plan['qbound']
    nch = len(chunks)

    with TileContext(nc) as tc:
        with tc.tile_pool(name="const", bufs=1) as cpool, \
             tc.tile_pool(name="qk", bufs=1) as qkpool, \
             tc.tile_pool(name="vt", bufs=1) as vtpool, \
             tc.tile_pool(name="diag", bufs=1) as dgpool, \
             tc.tile_pool(name="outsb", bufs=1) as opool, \
             tc.tile_pool(name="wo", bufs=1) as wop:

            idb = cpool.tile([128, 128], BF16)
            masks.make_identity(nc, idb[:])
            bqkv_sb = cpool.tile([128, NT], F32)
            nc.scalar.dma_start(bqkv_sb[:],
                                bqkv_in.ap().rearrange("(t p) -> p t", p=128))
            bsc = cpool.tile([128, NT], F32)
            nc.scalar.mul(bsc[:, 0:2], bqkv_sb[:, 0:2], 0.125)
            nc.scalar.copy(bsc[:, 2:NT], bqkv_sb[:, 2:NT])
            onesdiag = cpool.tile([128, 128], BF16)
            nc.gpsimd.memset(onesdiag[:], 0.0)
            nc.gpsimd.memset(onesdiag[0:64, 0:64], 1.0)
            nc.gpsimd.memset(onesdiag[64:128, 64:128], 1.0)

            wo_sb = [wop.tile([128, E], F32R, name=f"wo{hp}") for hp in range(2)]
            qT = [qkpool.tile([128, S], BF16, name=f"qT{hp}") for hp in range(2)]
            kT = [qkpool.tile([128, S], BF16, name=f"kT{hp}") for hp in range(2)]
            V = [vtpool.tile([128, (NB // 2) * D], BF16, name=f"V{h}")
                 for h in range(HPC)]
            kdiag = [dgpool.tile([128, NB * 128], BF16, name=f"kdiag{hp}")
                     for hp in range(2)]
            Vdiag = [dgpool.tile([128, NB * 128], BF16, name=f"Vdiag{hp}")
                     for hp in range(2)]
            outSB = [opool.tile([128, S], F32R, name=f"outSB{hp}") for hp in range(2)]

            with tc.tile_pool(name="sc_ps", bufs=3, space="PSUM") as sc_ps, \
                 tc.tile_pool(name="dn_ps", bufs=2, space="PSUM") as dn_ps, \
                 tc.tile_pool(name="ex", bufs=6) as expool, \
                 tc.tile_pool(name="rc", bufs=6) as rcpool, \
                 tc.tile_pool(name="at", bufs=nsplit + 8) as atpool:

                ex_saved = {}    # (hp, ci) -> (ex tile, ncols)
                at2_saved = {}   # (hp, ci) -> at2 tile
                mult_ctr = [0]
                dn_pools = [dn_ps]   # phase 2 appends a 3rd ring slot

                def emit_A(hp, ci):
                    """scores -> exp for one chunk."""
                    group = chunks[ci]
                    ncols = sum(n for _, _, _, n in group) * 64
                    spt = sc_ps.tile([128, 512], F32, tag="spt")
                    for gi, (pos, j, i0, n) in enumerate(group):
                        nc.tensor.matmul(
                            spt[:, pos * 64:(pos + n) * 64],
                            kdiag[hp][:, j * 128:(j + 1) * 128],
                            qT[hp][:, i0 * 64:(i0 + n) * 64],
                            start=(gi == 0), stop=(gi == len(group) - 1))
                    ex = expool.tile([128, 512], BF16, tag="ex")
                    nc.scalar.activation(ex[:, 0:ncols], spt[:, 0:ncols], AF.Exp)
                    ex_saved[(hp, ci)] = (ex, ncols)

                def emit_B(hp, ci):
                    """den -> recip -> mult for one chunk (needs A done)."""
                    ex, ncols = ex_saved.pop((hp, ci))
                    pool = (dn_pools[1] if len(dn_pools) > 1
                            and mult_ctr[0] % 3 == 2 else dn_pools[0])
                    dnb = pool.tile([128, 512], F32, tag="dnb")
                    nc.tensor.matmul(dnb[:, 0:ncols], onesdiag[:],
                                     ex[:, 0:ncols], start=True, stop=True)
                    # bf16 rec halves SBUF read traffic of the mult; the
                    # custom op computes in fp32 and casts on write (the
                    # fp32-only wrapper assert is about the INPUT bit trick)
                    rec = rcpool.tile([128, 512], BF16, tag="rec")
                    nc.vector._custom_dve(_RF, out=rec[:, 0:ncols],
                                          in0=dnb[:, 0:ncols], s0=_RC["s0"],
                                          s1=_RC["s1"], imm2=_RC["imm2"])
                    at2 = atpool.tile([128, 512], BF16, tag="at2")
                    k = mult_ctr[0] % 16
                    mult_ctr[0] += 1
                    if k in dve_mult:
                        nc.vector.tensor_tensor(at2[:, 0:ncols], ex[:, 0:ncols],
                                                rec[:, 0:ncols], ALU.mult)
                    else:
                        nc.gpsimd.tensor_tensor(at2[:, 0:ncols], ex[:, 0:ncols],
                                                rec[:, 0:ncols], ALU.mult)
                    at2_saved[(hp, ci)] = at2

                def emit_C(hp, ci, otp, ibase):
                    at2 = at2_saved.pop((hp, ci))
                    for pos, j, i0, n in chunks[ci]:
                        st, sp = flags[(j, i0)]
                        o0 = (i0 - ibase) * 64
                        nc.tensor.matmul(
                            otp[:, o0:o0 + n * 64],
                            Vdiag[hp][:, j * 128:(j + 1) * 128],
                            at2[:, pos * 64:(pos + n) * 64],
                            start=st, stop=sp)

                # ---- QKV (f32-free: bf16 weights-stationary) ----------------
                qkv_scale = [0.125, 0.125, 1.0, 1.0, 1.0, 1.0]
                with tc.tile_pool(name="xin", bufs=1) as xpool, \
                     tc.tile_pool(name="wq", bufs=1) as wpool, \
                     tc.tile_pool(name="qkv_ps", bufs=2, space="PSUM") as qkv_ps, \
                     tc.tile_pool(name="tr_ps", bufs=1, space="PSUM") as tr_ps:
                    vT = [xpool.tile([128, S], BF16, name=f"vT{hp}")
                          for hp in range(2)]
                    qkv_dst = [qT[0], qT[1], kT[0], kT[1], vT[0], vT[1]]
                    wsb = [wpool.tile([128, 3 * HPC * D], BF16, name=f"w{k}")
                           for k in range(KT)]
                    xsc = [xpool.tile([128, KT, 512], BF16, name=f"xsc{sc}")
                           for sc in range(SC)]
                    dma_engs = (nc.sync, nc.scalar)   # hw-DGE queues only
                    # weights first (consumed by every block), then x slices
                    # per k-tile in consumption order.
                    for k in range(KT):
                        dma_engs[k % 2].dma_start(
                            wsb[k][:], wqkv_in.ap()[k * 128:(k + 1) * 128, :])
                    xT_v = xT_in.ap().rearrange("(k p) s -> p k s", p=128)
                    for sc in range(SC):
                        for k in range(KT):
                            dma_engs[(sc * KT + k) % 2].dma_start(
                                xsc[sc][:, k, :],
                                xT_v[:, k, sc * 512:(sc + 1) * 512])
                    # big diag zero-fills: after the DMA triggers so they
                    # don't delay the x/w streams
                    nc.vector.memset(kdiag[0][:], 0.0)
                    nc.gpsimd.memset(kdiag[1][:], 0.0)
                    nc.vector.memset(Vdiag[0][:], 0.0)
                    nc.gpsimd.memset(Vdiag[1][:], 0.0)

                    def emit_qkv_block(t, sc):
                        pt = qkv_ps.tile([128, 512], F32, tag="qkvmm")
                        for k in range(KT):
                            nc.tensor.matmul(
                                pt[:],
                                wsb[k][:, t * 128:(t + 1) * 128],
                                xsc[sc][:, k, :],
                                start=(k == 0), stop=(k == KT - 1))
                        nc.scalar.activation(
                            qkv_dst[t][:, sc * 512:(sc + 1) * 512], pt[:],
                            AF.Identity, bias=bsc[:, t:t + 1],
                            scale=qkv_scale[t])

                    def emit_kdiag(hp, quarter=None):
                        kd = kdiag[hp][:, :]
                        for (p0, c0) in ((0, 0), (64, 64)):
                            dst = kd[p0:p0 + 64, :].rearrange(
                                "p (j c) -> p j c", c=128)[:, :, c0:c0 + 64]
                            src = kT[hp][p0:p0 + 64, :].rearrange(
                                "p (j c) -> p j c", c=64)
                            if quarter is None:
                                nc.sync.dma_start(dst, src)
                            else:
                                q8 = quarter * 8
                                nc.sync.dma_start(dst[:, q8:q8 + 8, :],
                                                  src[:, q8:q8 + 8, :])

                    def emit_vprep(vp):
                        for c4 in range(0, NB // 2, 4):
                            tp = tr_ps.tile([128, 512], BF16, tag="vtr")
                            for u in range(4):
                                c = c4 + u
                                nc.tensor.transpose(
                                    tp[:, u * 128:(u + 1) * 128],
                                    vT[vp][:, c * 128:(c + 1) * 128], idb[:])
                            for lh in range(2):
                                src = tp[:, 0:512].rearrange(
                                    "p (u x) -> p u x", x=128)[
                                    :, :, lh * 64:(lh + 1) * 64]
                                dst = V[2 * vp + lh][
                                    :, c4 * 64:(c4 + 4) * 64].rearrange(
                                    "p (u d) -> p u d", d=64)
                                if lh == 0:
                                    nc.scalar.copy(dst, src)
                                else:
                                    nc.vector.tensor_copy(dst, src)
                        vd = Vdiag[vp][:, :]
                        for lh in range(2):
                            h = 2 * vp + lh
                            pd, cd = (0, 0) if lh == 0 else (64, 64)
                            for par in range(2):
                                dst = vd[pd:pd + 64, :].rearrange(
                                    "p (c x) -> p c x", x=256)[
                                    :, :,
                                    par * 128 + cd:par * 128 + cd + 64]
                                src = V[h][par * 64:(par + 1) * 64, :].rearrange(
                                    "p (c d) -> p c d", d=64)
                                nc.sync.dma_start(dst, src)

                    # chunk ci eligible once its qT s-chunk (== its i-quarter)
                    # and kdiag quarters are written (subtile deps enforce
                    # correctness; this ordering only aids overlap)
                    nfront = min(nsplit, nch)
                    need_q = [max((i0 + n - 1) // 8 for _, _, i0, n in
                                  chunks[ci]) for ci in range(nfront)]
                    need_k = [max(j // 8 for _, j, _, _ in chunks[ci])
                              for ci in range(nfront)]
                    next_A = [0]
                    pend_B = []

                    def emit_eligible(sdone, kdone, cap):
                        # drain pending B's (their exp had >=1 block of PE
                        # time to complete), keeping one in flight, then
                        # emit new A's
                        done = 0
                        while len(pend_B) > 1 and done < cap:
                            emit_B(0, pend_B.pop(0))
                            done += 1
                        while (next_A[0] < nfront and done < cap
                               and need_q[next_A[0]] < sdone
                               and need_k[next_A[0]] < kdone):
                            emit_A(0, next_A[0])
                            pend_B.append(next_A[0])
                            next_A[0] += 1
                            done += 1

                    # sc-major order: all 6 projections of an x chunk run
                    # before the next chunk is needed, so PE never waits on
                    # a later x DMA while compute on arrived data remains
                    # hybrid order: q/k/vA per x chunk as it arrives (keeps
                    # PE fed during the DMA stream and unlocks fronts per
                    # quarter), then the pair-1 projections with fronts
                    # interleaved
                    for sc in range(SC):
                        emit_qkv_block(2, sc)
                        emit_kdiag(0, quarter=sc)
                        emit_eligible(sc, sc + 1, 4)
                        emit_qkv_block(0, sc)
                        emit_eligible(sc + 1, sc + 1, 8)
                        emit_qkv_block(4, sc)
                        emit_eligible(sc + 1, sc + 1, 4)
                    emit_vprep(0)
                    blocks_rest = [(t, sc) for t in (1, 3, 5)
                                   for sc in range(SC)]
                    bi_ = 0

                    def drain_block():
                        nonlocal bi_
                        t, sc = blocks_rest[bi_]
                        emit_qkv_block(t, sc)
                        bi_ += 1
                        if (t, sc) == (3, SC - 1):
                            emit_kdiag(1)
                        elif (t, sc) == (5, SC - 1):
                            emit_vprep(1)

                    while next_A[0] < nfront or pend_B or bi_ < len(blocks_rest):
                        emit_eligible(SC, SC, 3)
                        if bi_ < len(blocks_rest):
                            drain_block()
                        elif next_A[0] < nfront:
                            emit_A(0, next_A[0])
                            pend_B.append(next_A[0])
                            next_A[0] += 1
                        elif pend_B:
                            emit_B(0, pend_B.pop(0))

                for hp in range(2):
                    eng = nc.scalar if hp else nc.sync
                    eng.dma_start(wo_sb[hp][:],
                                  wo_in.ap()[hp * 128:(hp + 1) * 128, :])

                # ---- attention phase B: software-pipelined A/B/C ------------
                # flat schedule over (hp, ci); segment = one 1-bank otp pass
                sched = []
                seg_of = []
                seg_ibase = []
                seg_wo_q = []   # wo quarter unlocked when this seg flushes
                for hp in range(2):
                    for q in range(4):
                        sid = len(seg_ibase)
                        seg_ibase.append(q * 8)
                        seg_wo_q.append(q if hp == 1 else None)
                        for ci in range(qbound[q], qbound[q + 1]):
                            sched.append((hp, ci))
                            seg_of.append(sid)
                npos = len(sched)
                a_cur = [0]
                b_cur = [0]

                atcap = nsplit + 4   # at-pool bufs minus in-flight margin

                def ensure_A(upto):
                    # advance to `upto` unconditionally, then keep going
                    # while the ex ring and at pool have room -- keeps the
                    # front-end fed while pre-saved at2 chunks drain
                    while a_cur[0] < npos:
                        hp, ci = sched[a_cur[0]]
                        done = (hp, ci) in ex_saved or (hp, ci) in at2_saved
                        if not done:
                            if a_cur[0] > upto and (
                                    len(ex_saved) >= 5
                                    or len(at2_saved) >= atcap):
                                break
                            emit_A(hp, ci)
                        a_cur[0] += 1

                def ensure_B(upto):
                    # opportunistic B stays >=2 behind A so a fresh den
                    # never waits on an exp emitted in the same burst
                    lim = npos if a_cur[0] >= npos else a_cur[0] - 2
                    while b_cur[0] < min(lim, npos):
                        hp, ci = sched[b_cur[0]]
                        if (hp, ci) in ex_saved:
                            if b_cur[0] > upto and len(at2_saved) >= atcap:
                                break
                            emit_B(hp, ci)
                        b_cur[0] += 1

                # Wo blocks for i-quarter q need outSB[0] and outSB[1] cols
                # q*512:(q+1)*512, i.e. the flushes of segments q and 4+q --
                # emit them right after segment 4+q flushes so the Wo matmuls
                # fill PE while the saturated DVE/GPSIMD front-end drains.
                # The Wo PSUM tile shares the sc_ps ring (same tag/shape).
                with tc.tile_pool(name="yt", bufs=4) as ypool, \
                     tc.tile_pool(name="ot_ps", bufs=2, space="PSUM") as ot_ps, \
                     tc.tile_pool(name="dn2_ps", bufs=1, space="PSUM") as dn2_ps:
                    dn_pools.append(dn2_ps)   # den ring 2 -> 3 for phase 2

                    wo_pend = []

                    def emit_wo_block(st_):
                        yt = ypool.tile([128, E], BF16, tag="yt", name="yt")
                        for nchk in range(2):
                            pt = sc_ps.tile([128, 512], F32, tag="spt")
                            for hp in range(2):
                                nc.tensor.matmul(
                                    pt[:],
                                    outSB[hp][:, st_ * 128:(st_ + 1) * 128],
                                    wo_sb[hp][:, nchk * 512:(nchk + 1) * 512],
                                    start=(hp == 0), stop=(hp == 1))
                            nc.scalar.copy(
                                yt[:, nchk * 512:(nchk + 1) * 512], pt[:])
                        nc.sync.dma_start(
                            y_out.ap()[st_ * 128:(st_ + 1) * 128, :], yt[:])

                    def seg_done(seg, hp_prev):
                        # hp1 segments flush on DVE: ACT carries exp + the
                        # yt copies of the interleaved Wo blocks
                        _flush_otp(nc, outSB, otp, seg_ibase[seg], hp_prev,
                                   on_act=(seg_wo_q[seg] is None))
                        if seg_wo_q[seg] is not None:
                            q = seg_wo_q[seg]
                            wo_pend.extend(range(4 * q, 4 * q + 4))

                    otp = None
                    cur_seg = -1
                    for p in range(npos):
                        ensure_A(p + la)
                        ensure_B(p + lb)
                        if wo_pend and (p % 3 == 0 or len(at2_saved) < 5):
                            emit_wo_block(wo_pend.pop(0))
                        if seg_of[p] != cur_seg:
                            if otp is not None:
                                seg_done(cur_seg, sched[p - 1][0])
                            cur_seg = seg_of[p]
                            otp = ot_ps.tile([128, 512], F32, tag="otp")
                        hp, ci = sched[p]
                        emit_C(hp, ci, otp, seg_ibase[cur_seg])
                    seg_done(cur_seg, sched[npos - 1][0])
                    for st_ in wo_pend:
                        emit_wo_block(st_)

    nc.compile()
    return nc


def _flush_otp(nc, outSB, otp, ibase, hp, on_act=True):
    dst = outSB[hp][:, ibase * 64:ibase * 64 + 512]
    if on_act:
        nc.scalar.copy(dst, otp[:, 0:512])
    else:
        nc.vector.tensor_copy(dst, otp[:, 0:512])


# ---------------------------------------------------------------- entry point

def kernel(x, Wq, bq, Wk, bk, Wv, bv, Wo, bo, block_rows, block_cols):
    global LAST_RESULTS
    from concourse.bass_utils import run_bass_kernel_spmd
    import os

    x = np.asarray(x, dtype=np.float32)
    Wq, Wk, Wv, Wo = (np.asarray(a, dtype=np.float32) for a in (Wq, Wk, Wv, Wo))
    bq, bk, bv, bo = (np.asarray(a, dtype=np.float32) for a in (bq, bk, bv, bo))

    plan = _plan(block_rows, block_cols)
    nc = _build_program(plan)

    import ml_dtypes
    bf16 = ml_dtypes.bfloat16
    xT = [np.ascontiguousarray(x[b].T).astype(bf16) for b in range(B)]
    in_maps = []
    for c in range(NCORES):
        b, g = c // 4, c % 4
        cs = slice(g * HPC * D, (g + 1) * HPC * D)
        w_qkv = np.ascontiguousarray(
            np.concatenate([Wq[:, cs], Wk[:, cs], Wv[:, cs]], axis=1)).astype(bf16)
        b_qkv = np.ascontiguousarray(
            np.concatenate([bq[cs], bk[cs], bv[cs]]))
        w_o = np.ascontiguousarray(Wo[cs, :])
        in_maps.append(dict(xT_local=xT[b], w_qkv=w_qkv, b_qkv=b_qkv, w_o=w_o))

    trace = bool(int(os.environ.get("KERNEL_TRACE", "0")))
    res = run_bass_kernel_spmd(nc, in_maps, core_ids=list(range(NCORES)),
                               trace=trace)
    LAST_RESULTS = res

    y = np.zeros((B, S, E), dtype=np.float32)
    for c in range(NCORES):
        y[c // 4] += np.asarray(res.results[c]["y_partial"], dtype=np.float32)
    y += bo
    return y
